# revision 19
# baseline (speedup 1.0000x reference)
"""Multi-head self-attention Trainium2 kernel (8-core SPMD, full IO).

Problem: x:(2,2048,1024) f32; Wq/Wk/Wv/Wo:(1024,1024); bo:(1024,)
  out = softmax((xWq)(xWk)^T / 8) (xWv) reshaped @ Wo + bo

Sharding: data parallel on batch N=2 x tensor parallel on 16 heads in
4 groups of 4 heads.  Core c handles batch c//4, heads [4*(c%4), 4*(c%4)+4).
Each core computes a partial fc_out product (2048,1024) in bf16; the host
sums the 4 head-group partials per batch in f32 and adds the bias.

v3 design:
  - single fused software pipeline: projections are interleaved into the
    first attention block's m-loop so the scalar engine starts exp'ing
    early and the PE never idles (HAM stays warm).
  - m-loop is software-pipelined one deep: PE queue order is
    QK(m+1), PV(m), so the PE streams QK(m+1) while exp(m) runs on
    ACT/DVE instead of stalling at PV(m).
  - scores stay transposed S^T[k,q]; the ones column of V yields the
    softmax denominator in row 64 of the PV psum.
  - exp split across engines: ACT exact exp for 11/16 m-steps (all 16 in
    the projection-heavy first block), DVE Schraudolph bit-trick exp
    (i16 = S*A + B, bits are bf16) for the rest.  Full-pipeline rel err
    with 100% Schraudolph is 1.0e-2; this mix lands ~6.5e-3 (< 2e-2).
  - denominators: per q-chunk, the 4 head den rows bounce through DRAM,
    one packed [32,64] reciprocal, broadcast back; normalize fuses into
    the psum->SBUF staging; fc_out of chunk qc is emitted a full block
    later so its OT2 inputs are never on the critical path.
"""

import os

import numpy as np

import concourse.bass as bass
import concourse.tile as tile
from concourse import bacc, mybir
from concourse import bass_utils

F32 = mybir.dt.float32
BF16 = mybir.dt.bfloat16
I16 = mybir.dt.int16

EMBED = 1024
SEQ = 2048
NB = 2
HEADS = 16
D = 64
NCORES = 8
GROUPS = 4
HG = HEADS // GROUPS  # 4 heads per core
DG = HG * D  # 256 dims per core
KC = EMBED // 128  # 8 contraction chunks for projections
TCH = 512  # projection token chunk
NT = SEQ // TCH  # 4 chunks
QC = 512  # attention q-chunk
NM = SEQ // 128  # 16 k-chunks

# Schraudolph bf16 exp constants: i16 = trunc(S * A + B) are the bf16 bits
# of ~exp(S/8).  A = 128*log2(e)/8, B = 128*127 - sigma (+0.5 trunc comp).
SCHR_A = float(128.0 * 1.4426950408889634 / 8.0)
SCHR_B = float(128 * 127 - 7.0 + 0.5)

ACT_EXPS_STEADY = int(os.environ.get("MHA_ACT_EXPS", "11"))

LAST_RESULTS = None
_CACHED_NC = {}
_MM_DTYPE_NAME = f"bfloat16 + schraudolph (ACT {ACT_EXPS_STEADY}/16 steady)"


def build_nc():
    nc = bacc.Bacc("TRN2", target_bir_lowering=False, debug=False,
                   num_devices=NCORES)

    xT = nc.dram_tensor("xT", (EMBED, SEQ), BF16, kind="ExternalInput").ap()
    wq = nc.dram_tensor("wq", (EMBED, DG), BF16, kind="ExternalInput").ap()
    wk = nc.dram_tensor("wk", (EMBED, DG), BF16, kind="ExternalInput").ap()
    wv = nc.dram_tensor("wv", (EMBED, DG), BF16, kind="ExternalInput").ap()
    wo = nc.dram_tensor("wo", (DG, EMBED), BF16, kind="ExternalInput").ap()
    y = nc.dram_tensor("y", (SEQ, EMBED), BF16, kind="ExternalOutput").ap()
    # DRAM bounce buffers: SBUF sources can't be partition-broadcast by
    # DMA, DRAM sources can; den also bounces to repack for the reciprocal.
    den_dram = nc.dram_tensor("den_scratch", (NT, HG, QC), F32).ap()
    rden_dram = nc.dram_tensor("rden_scratch", (NT, HG, QC), F32).ap()

    xTr = xT.rearrange("(c p) s -> p c s", p=128)

    with tile.TileContext(nc) as tc:
        with (
            tc.tile_pool(name="weights", bufs=1) as wpool,
            tc.tile_pool(name="xpool", bufs=1) as xpool,
            tc.tile_pool(name="qk", bufs=1) as qkpool,
            tc.tile_pool(name="vpool", bufs=1) as vpool,
            tc.tile_pool(name="otpool", bufs=1) as otpool,
            tc.tile_pool(name="espool", bufs=4) as espool,
            tc.tile_pool(name="stage", bufs=4) as stpool,
            tc.tile_pool(name="stnp", bufs=4) as stnpool,
            tc.tile_pool(name="recp", bufs=2) as recpool,
            tc.tile_pool(name="rbc", bufs=4) as rbcpool,
            tc.tile_pool(name="ystage", bufs=3) as ypool,
            # sc bufs=3 gives the QK stream a 3-deep lookahead over the
            # exp WAR chain (the binding constraint at bufs=2: period =
            # QK + exp + 2 sem delays).  po only needs bufs=2 because its
            # tiles drain to SBUF immediately after each block.
            tc.tile_pool(name="psum_sc", bufs=3, space="PSUM") as scpool,
            tc.tile_pool(name="psum_po", bufs=2, space="PSUM") as popool,
        ):
            # ---- persistent SBUF tiles ----
            wq_sb = wpool.tile([128, KC, DG], BF16)
            wk_sb = wpool.tile([128, KC, DG], BF16)
            wv_sb = wpool.tile([128, KC, DG], BF16)
            wo_sb = wpool.tile([128, DG // 128, EMBED], BF16)

            xcs = [xpool.tile([128, KC, TCH], BF16, name=f"xc{t}", tag=f"xc{t}")
                   for t in range(NT)]
            QTs = [qkpool.tile([128, 2, TCH], BF16, name=f"qt{t}", tag=f"qt{t}")
                   for t in range(NT)]
            KTs = [qkpool.tile([128, 2, TCH], BF16, name=f"kt{t}", tag=f"kt{t}")
                   for t in range(NT)]
            Vs = [vpool.tile([128, 4, HG, D + 1], BF16, name=f"v{t}", tag=f"v{t}")
                  for t in range(NT)]
            OT2 = otpool.tile([128, 2, SEQ], BF16)

            # ---- input DMAs: only what the first projections need; the
            # rest is loaded just-in-time from the first block's pre-emits
            # so the first chunk doesn't compete for DMA bandwidth.  Spread
            # across trigger engines (separate DMA queues run in parallel).
            wk_r = wk.rearrange("(c p) n -> p c n", p=128)
            nc.sync.dma_start(out=wk_sb[:, 0:KC // 2, :], in_=wk_r[:, 0:KC // 2, :])
            nc.sync.dma_start(out=wk_sb[:, KC // 2:, :], in_=wk_r[:, KC // 2:, :])
            nc.scalar.dma_start(out=xcs[0][:, 0:KC // 2, :],
                                in_=xTr[:, 0:KC // 2, 0:TCH])
            nc.scalar.dma_start(out=xcs[0][:, KC // 2:, :],
                                in_=xTr[:, KC // 2:, 0:TCH])
            nc.gpsimd.dma_start(out=wq_sb, in_=wq.rearrange("(c p) n -> p c n", p=128))
            nc.gpsimd.dma_start(out=wv_sb, in_=wv.rearrange("(c p) n -> p c n", p=128))

            def load_x(t):
                nc.gpsimd.dma_start(out=xcs[t], in_=xTr[:, :, t * TCH:(t + 1) * TCH])

            def load_wo():
                nc.gpsimd.dma_start(out=wo_sb, in_=wo.rearrange("(c p) n -> p c n", p=128))

            for t in range(NT):
                nc.vector.memset(Vs[t][:, :, :, D:D + 1], 1.0)

            # ---- projection emitters (PE + DVE copy) ----
            def proj_qk(wsb, dst, t):
                for mt in range(2):
                    ps = scpool.tile([128, 2, QC], F32, name="pp", tag="sc")
                    for kc in range(KC):
                        nc.tensor.matmul(
                            ps[:, 0, :],
                            wsb[:, kc, mt * 128:(mt + 1) * 128],
                            xcs[t][:, kc, :],
                            start=(kc == 0), stop=(kc == KC - 1),
                        )
                    nc.vector.tensor_copy(out=dst[t][:, mt, :], in_=ps[:, 0, :])

            def proj_v(t):
                for ti in range(TCH // 128):
                    ps = scpool.tile([128, 2, QC], F32, name="pv", tag="sc")
                    for kc in range(KC):
                        nc.tensor.matmul(
                            ps[:, 0, 0:DG],
                            xcs[t][:, kc, ti * 128:(ti + 1) * 128],
                            wv_sb[:, kc, :],
                            start=(kc == 0), stop=(kc == KC - 1),
                        )
                    nc.vector.tensor_copy(
                        out=Vs[t][:, ti, :, 0:D],
                        in_=ps[:, 0, 0:DG].rearrange("p (h d) -> p h d", h=HG))

            # ---- attention block (hm, qc), software-pipelined m-loop ----
            def attn_block(hm, qc, pre_emit, n_act):
                po = [popool.tile([D + 1, QC], F32, name=f"po{j}", tag="po")
                      for j in range(2)]

                def emit_qk(m):
                    sc = scpool.tile([128, 2, QC], F32, name="sc", tag="sc")
                    for j in range(2):
                        nc.tensor.matmul(
                            sc[:, j, :],
                            KTs[m // 4][j * D:(j + 1) * D, hm,
                                        (m % 4) * 128:(m % 4 + 1) * 128],
                            QTs[qc][j * D:(j + 1) * D, hm, :],
                            start=True, stop=True,
                        )
                    return sc

                def emit_exp(m, sc):
                    es = espool.tile([128, 2, QC], BF16, name="es", tag="es")
                    on_act = (m * n_act) // NM != ((m + 1) * n_act) // NM \
                        if n_act < NM else True
                    if on_act:
                        nc.scalar.activation(
                            out=es, in_=sc,
                            func=mybir.ActivationFunctionType.Exp,
                            scale=1.0 / np.sqrt(D),
                        )
                    else:
                        nc.vector.tensor_scalar(
                            es.bitcast(I16), sc, SCHR_A, SCHR_B,
                            mybir.AluOpType.mult, mybir.AluOpType.add,
                        )
                    return es

                def emit_pv(m, es):
                    for j in range(2):
                        nc.tensor.matmul(
                            po[j],
                            Vs[m // 4][:, m % 4, 2 * hm + j, :],
                            es[:, j, :],
                            start=(m == 0), stop=(m == NM - 1),
                        )

                # software pipeline 2 deep: PE order QK(m), PV(m-2) so PV
                # never reaches the queue head before its es is ready.
                scs, ess = {}, {}
                for m in range(NM):
                    for fn in pre_emit.get(m, ()):
                        fn()
                    scs[m] = emit_qk(m)
                    if m >= 1:
                        ess[m - 1] = emit_exp(m - 1, scs.pop(m - 1))
                    if m >= 2:
                        emit_pv(m - 2, ess.pop(m - 2))
                ess[NM - 1] = emit_exp(NM - 1, scs.pop(NM - 1))
                emit_pv(NM - 2, ess.pop(NM - 2))
                emit_pv(NM - 1, ess.pop(NM - 1))
                return po

            # ---- post-block: stage O^T + den row out ----
            def post_block(hm, qc, po):
                strs = []
                for j in range(2):
                    h = 2 * hm + j
                    st = stpool.tile([D + 1, QC], F32, name="st", tag="st")
                    nc.vector.tensor_copy(out=st, in_=po[j])
                    nc.sync.dma_start(out=den_dram[qc, h:h + 1, :],
                                      in_=st[D:D + 1, :])
                    strs.append(st)
                return strs

            # ---- per-(qc,hm) denominator: packed recip + broadcast + norm ----
            def den_hm(qc, hm, strs):
                qs = slice(qc * QC, (qc + 1) * QC)
                rsm = recpool.tile([16, QC // 8], F32, name="rsm", tag="rsm")
                den_r = den_dram[qc, 2 * hm:2 * hm + 2].rearrange(
                    "h (a b) -> (h a) b", a=8)
                rden_r = rden_dram[qc, 2 * hm:2 * hm + 2].rearrange(
                    "h (a b) -> (h a) b", a=8)
                nc.sync.dma_start(out=rsm, in_=den_r)
                nc.vector.reciprocal(out=rsm, in_=rsm)
                nc.sync.dma_start(out=rden_r, in_=rsm)
                for j in range(2):
                    h = 2 * hm + j
                    rb = rbcpool.tile([D, QC], F32, name="rb", tag="rb")
                    nc.sync.dma_start(
                        out=rb,
                        in_=rden_dram[qc, h:h + 1, :].to_broadcast((D, QC)))
                    stn = stnpool.tile([D, QC], BF16, name="stn", tag="stn")
                    # DVE, not gpsimd: gpsimd's periodic dge_drains (~2.4us)
                    # stall these muls right when the tail needs them
                    nc.vector.tensor_mul(stn, strs[j][0:D, :], rb)
                    nc.sync.dma_start(
                        out=OT2[j * D:(j + 1) * D, hm, qs], in_=stn)

            # ---- fc_out for one token tile (128 tokens, all 1024 cols) ----
            def fc_tt(qc, ti):
                tt = qc * (QC // 128) + ti
                ps = scpool.tile([128, 2, QC], F32, name="fc", tag="sc")
                for nch in range(2):
                    for hm in range(2):
                        nc.tensor.matmul(
                            ps[:, nch, :],
                            OT2[:, hm, tt * 128:(tt + 1) * 128],
                            wo_sb[:, hm, nch * 512:(nch + 1) * 512],
                            start=(hm == 0), stop=(hm == 1),
                        )
                ys = ypool.tile([128, 2, QC], BF16, name="ys", tag="ys")
                nc.vector.tensor_copy(out=ys, in_=ps)
                nc.sync.dma_start(
                    out=y[tt * 128:(tt + 1) * 128, :].rearrange(
                        "p (c n) -> p c n", c=2),
                    in_=ys)

            # ---- emission schedule ----
            proj_qk(wk_sb, KTs, 0)
            proj_qk(wq_sb, QTs, 0)
            proj_v(0)

            staged = {}  # (qc, hm) -> strs (2 staged O^T tiles)
            for qc in range(NT):
                for hm in range(2):
                    pre = {}
                    if qc == 0 and hm == 0:
                        for tp in range(1, NT):
                            pre[4 * tp] = [
                                (lambda t=tp: proj_qk(wk_sb, KTs, t)),
                                (lambda t=tp: proj_v(t)),
                            ]
                            # JIT x-chunk load, one proj-chunk ahead
                            pre.setdefault(4 * (tp - 1), []).insert(
                                0, (lambda t=tp: load_x(t)))
                        pre.setdefault(8, []).append(load_wo)
                        n_act = NM
                    else:
                        n_act = ACT_EXPS_STEADY
                    if qc == 0 and hm == 1:
                        pre[0] = [(lambda: proj_qk(wq_sb, QTs, 1))]
                    if hm == 1:
                        # den of this qc's first half, one block later
                        pre.setdefault(2, []).append(
                            (lambda q=qc: den_hm(q, 0, staged.pop((q, 0)))))
                    if qc >= 1 and hm == 0:
                        if qc + 1 < NT:
                            pre[0] = [(lambda t=qc + 1: proj_qk(wq_sb, QTs, t))]
                        pre.setdefault(2, []).append(
                            (lambda q=qc - 1: den_hm(q, 1, staged.pop((q, 1)))))
                    if qc >= 1 and hm == 1:
                        # fc_out of the previous q-chunk, a full block after
                        # its OT2 rows were staged
                        for i in range(4):
                            pre.setdefault(4 + 3 * i, []).append(
                                (lambda q=qc - 1, i=i: fc_tt(q, i)))
                    po = attn_block(hm, qc, pre, n_act)
                    staged[(qc, hm)] = post_block(hm, qc, po)

            den_hm(NT - 1, 1, staged.pop((NT - 1, 1)))
            for i in range(4):
                fc_tt(NT - 1, i)

    nc.compile()
    return nc


def shard_inputs(x, Wv, Wk, Wq, Wo):
    import ml_dtypes
    wire = ml_dtypes.bfloat16
    in_maps = []
    for c in range(NCORES):
        n, g = divmod(c, GROUPS)
        cols = slice(g * DG, (g + 1) * DG)
        in_maps.append({
            "xT": np.ascontiguousarray(np.asarray(x[n], np.float32).T).astype(wire),
            "wq": np.ascontiguousarray(np.asarray(Wq, np.float32)[:, cols]).astype(wire),
            "wk": np.ascontiguousarray(np.asarray(Wk, np.float32)[:, cols]).astype(wire),
            "wv": np.ascontiguousarray(np.asarray(Wv, np.float32)[:, cols]).astype(wire),
            "wo": np.ascontiguousarray(np.asarray(Wo, np.float32)[cols, :]).astype(wire),
        })
    return in_maps


def kernel(x, Wv, Wk, Wq, Wo, bo):
    global LAST_RESULTS
    x = np.asarray(x, np.float32)
    in_maps = shard_inputs(x, Wv, Wk, Wq, Wo)

    if "nc" not in _CACHED_NC:
        _CACHED_NC["nc"] = build_nc()
    nc = _CACHED_NC["nc"]

    trace = os.environ.get("MHA_TRACE", "0") == "1"
    res = bass_utils.run_bass_kernel_spmd(
        nc, in_maps, core_ids=list(range(NCORES)), trace=trace)
    LAST_RESULTS = res

    bo = np.asarray(bo, np.float32)
    out = np.empty((NB, SEQ, EMBED), np.float32)
    for n in range(NB):
        acc = res.results[n * GROUPS]["y"].astype(np.float32)
        for g in range(1, GROUPS):
            acc = acc + res.results[n * GROUPS + g]["y"].astype(np.float32)
        out[n] = acc + bo[None, :]
    return out
